# revision 10
# baseline (speedup 1.0000x reference)
"""GroupSort(2) Trainium2 Bass kernel.

The reference module
    diff = relu(w_diff @ x);  out = x + w_expand @ diff
with the fixed pair-difference weights is algebraically a pairwise sort:
    out[2k]   = min(x[2k], x[2k+1])
    out[2k+1] = max(x[2k], x[2k+1])
so the kernel is pure elementwise min/max — no matmuls.

Sharding: pure data parallel, batch 32 -> 8 cores x 4.
Per-core layout: x shard [4, 256, 64, 64] viewed as [4, 128, 2, 4096]
(channel pair k lives on partition k; even/odd members side by side).

The kernel is chip-HBM-bound: 8 cores x (16 MiB in + 16 MiB out) =
256 MiB at ~2.9 TB/s ~= 90 us.  The raw-bass pipeline below keeps the
single SP DGE queue saturated (loads run ahead of DVE-gated stores via
descriptor-attached sem waits) and minimizes preamble/epilogue time.
"""

import numpy as np

import bass_rust
import concourse.mybir as mybir
from concourse.bass import Bass
from concourse.tile import TileContext
from concourse.bass_utils import run_bass_kernel_spmd

N_CORES = 8
B, C, H, W = 32, 256, 64, 64
BS = B // N_CORES          # batches per core
P = 128                    # channel pairs -> SBUF partitions
HW = H * W                 # 4096
K = 2048                   # hw chunk per tile
NCHUNK = BS * (HW // K)    # 8
B_IN = 4                   # tin slots
B_OUT = 4                  # tout slots
LAG = 2                    # stores trail loads by LAG chunks in the queue
DT = mybir.dt.float32

_nc_cache = {}


def _build_raw():
    nc = Bass()
    x = nc.declare_dram_parameter("x", [BS, P, 2, HW], DT, isOutput=False)
    out = nc.declare_dram_parameter("out", [BS, P, 2, HW], DT, isOutput=True)

    def src(i):
        b, jj = divmod(i, HW // K)
        return x[b, :, :, jj * K : (jj + 1) * K]

    def dst(i):
        b, jj = divmod(i, HW // K)
        return out[b, :, :, jj * K : (jj + 1) * K]

    import contextlib

    with contextlib.ExitStack() as stack:
        block = stack.enter_context(nc.Block())
        # Per-chunk completion sems: DMA slice completions from the 16
        # HWDGE engines interleave across transfers, so one shared counter
        # cannot order chunk boundaries (race detector rightly objects).
        ld = [stack.enter_context(nc.semaphore(f"ld{i}")) for i in range(NCHUNK)]
        st = [stack.enter_context(nc.semaphore(f"st{i}")) for i in range(NCHUNK)]
        dv_sem = stack.enter_context(nc.semaphore("dv_sem"))
        tin = stack.enter_context(nc.sbuf_tensor("tin", [P, B_IN, 2, K], DT))
        tout = stack.enter_context(nc.sbuf_tensor("tout", [P, B_OUT, 2, K], DT))

        @block.sync
        def _(sync):
            def store(si):
                ins = sync.dma_start(out=dst(si), in_=tout[:, si % B_OUT])
                ins._wait_ge(dv_sem, si + 1)     # min+max of chunk si done
                ins.then_inc(st[si], 16)

            for i in range(NCHUNK):
                ins = sync.dma_start(out=tin[:, i % B_IN], in_=src(i))
                if i >= B_IN:
                    # slot reuse: DVE must have consumed chunk i-B_IN
                    ins._wait_ge(dv_sem, i - B_IN + 1)
                ins.then_inc(ld[i], 16)
                if i - LAG >= 0:
                    store(i - LAG)
            for si in range(NCHUNK - LAG, NCHUNK):
                store(si)
            for si in range(NCHUNK):
                sync.wait_ge(st[si], 16)

        @block.vector
        def _(vector):
            for i in range(NCHUNK):
                if i >= B_OUT:
                    # tout slot reuse: store of chunk i-B_OUT finished
                    vector.wait_ge(st[i - B_OUT], 16)
                ins = vector.tensor_tensor(
                    out=tout[:, i % B_OUT, 0],
                    in0=tin[:, i % B_IN, 0], in1=tin[:, i % B_IN, 1],
                    op=mybir.AluOpType.min,
                )
                ins._wait_ge(ld[i], 16)
                vector.tensor_tensor(
                    out=tout[:, i % B_OUT, 1],
                    in0=tin[:, i % B_IN, 0], in1=tin[:, i % B_IN, 1],
                    op=mybir.AluOpType.max,
                ).then_inc(dv_sem, 1)

    bass_rust.generate_event_semaphores(nc)
    nc.finalize()
    return nc


def _build_tile():
    nc = Bass()
    x = nc.declare_dram_parameter("x", [BS, P, 2, HW], DT, isOutput=False)
    out = nc.declare_dram_parameter("out", [BS, P, 2, HW], DT, isOutput=True)
    with TileContext(nc) as tc:
        with (
            tc.tile_pool(name="pin", bufs=4) as pin,
            tc.tile_pool(name="pout", bufs=3) as pout,
        ):
            for b in range(BS):
                for j in range(0, HW, K):
                    tin = pin.tile([P, 2, K], DT)
                    tout = pout.tile([P, 2, K], DT)
                    nc.sync.dma_start(out=tin, in_=x[b, :, :, j : j + K])
                    nc.vector.tensor_tensor(
                        out=tout[:, 0, :], in0=tin[:, 0, :], in1=tin[:, 1, :],
                        op=mybir.AluOpType.min,
                    )
                    nc.vector.tensor_tensor(
                        out=tout[:, 1, :], in0=tin[:, 0, :], in1=tin[:, 1, :],
                        op=mybir.AluOpType.max,
                    )
                    nc.sync.dma_start(out=out[b, :, :, j : j + K], in_=tout)
    # TRN2 allows at most one sync-wait per instruction; Tile can attach
    # several (load sem + slot-release sem). Split the excess onto
    # InstEventSemaphores or neuronxcc codegen rejects the TensorTensors.
    bass_rust.generate_event_semaphores(nc)
    nc.finalize()
    return nc


def _build(variant="raw"):
    if variant not in _nc_cache:
        _nc_cache[variant] = _build_raw() if variant == "raw" else _build_tile()
    return _nc_cache[variant]


def _run(x, trace=False, variant="raw", **kwargs):
    nc = _build(variant)
    xs = np.ascontiguousarray(np.asarray(x, dtype=np.float32)).reshape(
        N_CORES, BS, P, 2, HW
    )
    in_maps = [{"x": xs[i]} for i in range(N_CORES)]
    res = run_bass_kernel_spmd(
        nc, in_maps, core_ids=list(range(N_CORES)), trace=trace, **kwargs
    )
    out = np.stack([r["out"] for r in res.results], axis=0).reshape(B, C, H, W)
    return out, res


def kernel(x, **_unused_weights):
    out, _ = _run(x)
    return out


# revision 11
# speedup vs baseline: 1.1272x; 1.1272x over previous
"""GroupSort(2) Trainium2 Bass kernel.

The reference module
    diff = relu(w_diff @ x);  out = x + w_expand @ diff
with the fixed pair-difference weights is algebraically a pairwise sort:
    out[2k]   = min(x[2k], x[2k+1])
    out[2k+1] = max(x[2k], x[2k+1])
so the kernel is pure elementwise min/max — no matmuls.

Sharding: pure data parallel, batch 32 -> 8 cores x 4.
Per-core layout: x shard [4, 256, 64, 64] viewed as [4, 128, 2, 4096]
(channel pair k lives on partition k; even/odd members side by side).

The kernel is chip-HBM-bound: 8 cores x (16 MiB in + 16 MiB out) =
256 MiB at ~2.9 TB/s ~= 90 us.  The raw-bass pipeline below keeps the
single SP DGE queue saturated (loads run ahead of DVE-gated stores via
descriptor-attached sem waits) and minimizes preamble/epilogue time.
"""

import numpy as np

import bass_rust
import concourse.mybir as mybir
from concourse.bass import Bass
from concourse.tile import TileContext
from concourse.bass_utils import run_bass_kernel_spmd

N_CORES = 8
B, C, H, W = 32, 256, 64, 64
BS = B // N_CORES          # batches per core
P = 128                    # channel pairs -> SBUF partitions
HW = H * W                 # 4096
K = 2048                   # hw chunk per tile
NCHUNK = BS * (HW // K)    # 8
B_IN = 3                   # tin slots
B_OUT = 4                  # tout slots
LAG = 2                    # stores trail loads by LAG chunks in the queue
DT = mybir.dt.float32

_nc_cache = {}


def _build_raw():
    nc = Bass()
    x = nc.declare_dram_parameter("x", [BS, P, 2, HW], DT, isOutput=False)
    out = nc.declare_dram_parameter("out", [BS, P, 2, HW], DT, isOutput=True)

    def src(i):
        b, jj = divmod(i, HW // K)
        return x[b, :, :, jj * K : (jj + 1) * K]

    def dst(i):
        b, jj = divmod(i, HW // K)
        return out[b, :, :, jj * K : (jj + 1) * K]

    import contextlib

    with contextlib.ExitStack() as stack:
        block = stack.enter_context(nc.Block())
        # Per-chunk completion sems: DMA slice completions from the 16
        # HWDGE engines interleave across transfers, so one shared counter
        # cannot order chunk boundaries (race detector rightly objects).
        ld = [stack.enter_context(nc.semaphore(f"ld{i}")) for i in range(NCHUNK)]
        st = [stack.enter_context(nc.semaphore(f"st{i}")) for i in range(NCHUNK)]
        dv_sem = stack.enter_context(nc.semaphore("dv_sem"))
        tin = stack.enter_context(nc.sbuf_tensor("tin", [P, B_IN, 2, K], DT))
        tout = stack.enter_context(nc.sbuf_tensor("tout", [P, B_OUT, 2, K], DT))

        @block.sync
        def _(sync):
            def store(si):
                ins = sync.dma_start(out=dst(si), in_=tout[:, si % B_OUT])
                ins._wait_ge(dv_sem, si + 1)     # min+max of chunk si done
                ins.then_inc(st[si], 16)

            for i in range(NCHUNK):
                if i - LAG >= 0:
                    store(i - LAG)
                ins = sync.dma_start(out=tin[:, i % B_IN], in_=src(i))
                if i >= B_IN:
                    # slot reuse: DVE must have consumed chunk i-B_IN
                    ins._wait_ge(dv_sem, i - B_IN + 1)
                ins.then_inc(ld[i], 16)
            for si in range(NCHUNK - LAG, NCHUNK):
                store(si)
            for si in range(NCHUNK):
                sync.wait_ge(st[si], 16)

        @block.vector
        def _(vector):
            for i in range(NCHUNK):
                if i >= B_OUT:
                    # tout slot reuse: store of chunk i-B_OUT finished
                    vector.wait_ge(st[i - B_OUT], 16)
                ins = vector.tensor_tensor(
                    out=tout[:, i % B_OUT, 0],
                    in0=tin[:, i % B_IN, 0], in1=tin[:, i % B_IN, 1],
                    op=mybir.AluOpType.min,
                )
                ins._wait_ge(ld[i], 16)
                vector.tensor_tensor(
                    out=tout[:, i % B_OUT, 1],
                    in0=tin[:, i % B_IN, 0], in1=tin[:, i % B_IN, 1],
                    op=mybir.AluOpType.max,
                ).then_inc(dv_sem, 1)

    bass_rust.generate_event_semaphores(nc)
    nc.finalize()
    return nc


def _build_tile():
    nc = Bass()
    x = nc.declare_dram_parameter("x", [BS, P, 2, HW], DT, isOutput=False)
    out = nc.declare_dram_parameter("out", [BS, P, 2, HW], DT, isOutput=True)
    with TileContext(nc) as tc:
        with (
            tc.tile_pool(name="pin", bufs=4) as pin,
            tc.tile_pool(name="pout", bufs=3) as pout,
        ):
            for b in range(BS):
                for j in range(0, HW, K):
                    tin = pin.tile([P, 2, K], DT)
                    tout = pout.tile([P, 2, K], DT)
                    nc.sync.dma_start(out=tin, in_=x[b, :, :, j : j + K])
                    nc.vector.tensor_tensor(
                        out=tout[:, 0, :], in0=tin[:, 0, :], in1=tin[:, 1, :],
                        op=mybir.AluOpType.min,
                    )
                    nc.vector.tensor_tensor(
                        out=tout[:, 1, :], in0=tin[:, 0, :], in1=tin[:, 1, :],
                        op=mybir.AluOpType.max,
                    )
                    nc.sync.dma_start(out=out[b, :, :, j : j + K], in_=tout)
    # TRN2 allows at most one sync-wait per instruction; Tile can attach
    # several (load sem + slot-release sem). Split the excess onto
    # InstEventSemaphores or neuronxcc codegen rejects the TensorTensors.
    bass_rust.generate_event_semaphores(nc)
    nc.finalize()
    return nc


def _build(variant="raw"):
    if variant not in _nc_cache:
        _nc_cache[variant] = _build_raw() if variant == "raw" else _build_tile()
    return _nc_cache[variant]


def _run(x, trace=False, variant="raw", **kwargs):
    nc = _build(variant)
    xs = np.ascontiguousarray(np.asarray(x, dtype=np.float32)).reshape(
        N_CORES, BS, P, 2, HW
    )
    in_maps = [{"x": xs[i]} for i in range(N_CORES)]
    res = run_bass_kernel_spmd(
        nc, in_maps, core_ids=list(range(N_CORES)), trace=trace, **kwargs
    )
    out = np.stack([r["out"] for r in res.results], axis=0).reshape(B, C, H, W)
    return out, res


def kernel(x, **_unused_weights):
    out, _ = _run(x)
    return out
